# revision 15
# baseline (speedup 1.0000x reference)
"""Multi-head attention (B=4, N=2048, DIM=768, H=12) on 8 TRN2 NeuronCores.

Sharding: core c handles batch b = c//2 and head group g = c%2 (6 heads each).
Each core computes qkv projection, attention and the partial output projection
for its 6 heads; the host sums the two partial projections per batch (row-split
of the projection over heads). proj bias is applied on even cores only (odd
cores get zeros).

On-device dataflow (per core):
  - x^T is provided by the host as [768, 2048]; all matmuls run with the
    contraction on partitions.
  - Q^T / K^T are produced in [head_dim, n] layout, two heads packed per
    128-partition tile; scores are computed transposed (keys on partitions,
    queries on the free dim) so exp(scores) tiles feed the PV matmul directly
    with no transposes.
  - V is produced in natural [n, head_dim] layout, interleaved with a ones
    column per head; the ones column makes the PV matmul also accumulate the
    softmax denominator (row 64 of its PSUM output).
  - Softmax skips the max subtraction: scores here are ~N(0, 0.31) so exp
    needs no stabilization; matches jax softmax to fp32r/fp16 precision.
  - Normalization: PV PSUM is copied to SBUF immediately (releases the PSUM
    bank for the next tile), reciprocal_approx_fast on the denominators,
    broadcast across partitions via a K=2 selector matmul, multiply on DVE.
  - Program order interleaves the next pair's Q/K projection (and the output
    projection) into the ACT-bound attention stream so the PE stays busy and
    the HAM clock-gate keeps the PE at 2.4 GHz.
  - Matmuls run in float32r (1 cycle/row at N>=256, ~1.6e-4 rel err);
    probabilities and V in fp16 (~3e-4 contribution).
"""
import os
import numpy as np
from contextlib import ExitStack

import concourse.bass as bass
import concourse.tile as tile
from concourse import bacc, mybir
from concourse.bass_utils import run_bass_kernel_spmd

F32 = mybir.dt.float32
F32R = mybir.dt.float32r
F16 = mybir.dt.float16

B, N, DIM = 4, 2048, 768
H, HD = 12, 64
SCALE = HD ** -0.5
HPC = 6            # heads per core
NPAIR = 3          # head pairs per core
NJ = N // 128      # 16 key tiles
NQ5 = N // 512     # 4 query tiles of 512
JB = 2             # j-tiles per exp batch

_NC_CACHE = {}
LAST_EXEC_TIME_NS = None


def _build_nc():
    nc = bacc.Bacc("TRN2", target_bir_lowering=False, num_devices=1)

    xt_d = nc.declare_dram_parameter("xt", [DIM, N], F32R, isOutput=False)
    wq_d = nc.declare_dram_parameter("wq", [DIM, 384], F32R, isOutput=False)
    wk_d = nc.declare_dram_parameter("wk", [DIM, 384], F32R, isOutput=False)
    wv_d = nc.declare_dram_parameter("wv", [DIM, 384], F32R, isOutput=False)
    bq_d = nc.declare_dram_parameter("bq", [384], F32, isOutput=False)
    bk_d = nc.declare_dram_parameter("bk", [384], F32, isOutput=False)
    bv_d = nc.declare_dram_parameter("bv", [1, 384], F32R, isOutput=False)
    pw_d = nc.declare_dram_parameter("pw", [384, DIM], F32R, isOutput=False)
    pb_d = nc.declare_dram_parameter("pb", [1, DIM], F32R, isOutput=False)
    sel_d = nc.declare_dram_parameter("sel", [2, 128], F32, isOutput=False)
    ones_d = nc.declare_dram_parameter("ones1", [1, 128], F32R, isOutput=False)
    out_d = nc.declare_dram_parameter("out", [N, DIM], F32, isOutput=True)

    with tile.TileContext(nc) as tc, ExitStack() as ctx:
        consts = ctx.enter_context(tc.tile_pool(name="consts", bufs=1))
        xt_pool = ctx.enter_context(tc.tile_pool(name="xtp", bufs=2))
        big = ctx.enter_context(tc.tile_pool(name="big", bufs=1))
        pt_pool = ctx.enter_context(tc.tile_pool(name="ptp", bufs=4))
        pvs_pool = ctx.enter_context(tc.tile_pool(name="pvsp", bufs=2))
        small = ctx.enter_context(tc.tile_pool(name="small", bufs=2))
        outp = ctx.enter_context(tc.tile_pool(name="outp", bufs=3))
        st_pool = ctx.enter_context(tc.tile_pool(name="stp", bufs=1, space="PSUM"))
        pv_pool = ctx.enter_context(tc.tile_pool(name="pvp", bufs=1, space="PSUM"))
        scr_pool = ctx.enter_context(tc.tile_pool(name="scrp", bufs=2, space="PSUM"))

        # ---- constants ----
        wq_sb = consts.tile([128, 6, 384], F32R)
        wk_sb = consts.tile([128, 6, 384], F32R)
        wv_sb = consts.tile([128, 6, 384], F32R)
        nc.sync.dma_start(wq_sb[:], wq_d.rearrange("(co ci) m -> ci co m", ci=128))
        nc.sync.dma_start(wk_sb[:], wk_d.rearrange("(co ci) m -> ci co m", ci=128))
        nc.sync.dma_start(wv_sb[:], wv_d.rearrange("(co ci) m -> ci co m", ci=128))
        pw_sb = consts.tile([128, 3, DIM], F32R)
        nc.sync.dma_start(pw_sb[:], pw_d.rearrange("(ko ki) o -> ki ko o", ki=128))
        bq_sb = consts.tile([128, 3], F32)
        bk_sb = consts.tile([128, 3], F32)
        nc.sync.dma_start(bq_sb[:], bq_d.rearrange("(po pi) -> pi po", pi=128))
        nc.sync.dma_start(bk_sb[:], bk_d.rearrange("(po pi) -> pi po", pi=128))
        bv1 = consts.tile([1, 384], F32R)
        pb1 = consts.tile([1, DIM], F32R)
        nc.sync.dma_start(bv1[:], bv_d[:])
        nc.sync.dma_start(pb1[:], pb_d[:])
        sel = consts.tile([2, 128], F32)
        ones1 = consts.tile([1, 128], F32R)
        nc.sync.dma_start(sel[:], sel_d[:])
        nc.sync.dma_start(ones1[:], ones_d[:])

        # broadcast bv/pb across partitions via K=1 ones matmul
        bv_bc = consts.tile([128, 384], F32)
        pb_bc = consts.tile([128, DIM], F32)
        bv_ps = scr_pool.tile([128, 512], F32, name="scr")
        nc.tensor.matmul(bv_ps[:, 0:384], ones1[:], bv1[:], start=True, stop=True)
        nc.vector.tensor_copy(out=bv_bc[:], in_=bv_ps[:, 0:384])
        for oh in range(2):
            pb_ps = scr_pool.tile([128, 512], F32, name="scr")
            nc.tensor.matmul(pb_ps[:, 0:384], ones1[:], pb1[:, bass.ts(oh, 384)],
                             start=True, stop=True)
            nc.vector.tensor_copy(out=pb_bc[:, bass.ts(oh, 384)], in_=pb_ps[:, 0:384])

        qt_pairs = [big.tile([128, N], F16, name=f"qt{p}") for p in range(NPAIR)]
        kt_pairs = [big.tile([128, N], F16, name=f"kt{p}") for p in range(NPAIR)]
        v_sb = big.tile([128, NJ, HPC * 65], F16)
        nc.vector.memset(v_sb[:], 1.0)  # ones columns (at h*65+64) survive
        at_pairs = [big.tile([128, N], F32R, name=f"at{p}") for p in range(NPAIR)]

        def qk_chunk_gen(p, nt):
            """Q and K projection for pair p on the 512-wide n-chunk nt.
            Generator yielding between small pieces so the emission can be
            spread through an ACT-bound attention block. Q and K matmuls are
            interleaved so consecutive matmuls hit different PSUM banks."""
            ns_ = bass.ts(nt, 512)
            xt_t = load_xt(nt)
            qp = scr_pool.tile([128, 512], F32, name="scr")
            for ci in range(6):
                nc.tensor.matmul(qp[:], wq_sb[:, ci, bass.ts(p, 128)],
                                 xt_t[:, ci, :], start=(ci == 0), stop=(ci == 5))
                if ci == 2:
                    yield
            nc.vector.tensor_scalar_add(qt_pairs[p][:, ns_], qp[:], bq_sb[:, p:p + 1])
            yield
            kp = scr_pool.tile([128, 512], F32, name="scr")
            for ci in range(6):
                nc.tensor.matmul(kp[:], wk_sb[:, ci, bass.ts(p, 128)],
                                 xt_t[:, ci, :], start=(ci == 0), stop=(ci == 5))
                if ci == 2:
                    yield
            nc.vector.tensor_scalar_add(kt_pairs[p][:, ns_], kp[:], bk_sb[:, p:p + 1])
            yield

        def load_xt(nt):
            xt_t = xt_pool.tile([128, 6, 512], F32R, name="xt_t")
            nc.sync.dma_start(
                xt_t[:], xt_d.rearrange("(co ci) n -> ci co n", ci=128)[:, :, bass.ts(nt, 512)]
            )
            return xt_t

        def v_chunk(nt):
            xt_t = load_xt(nt)
            for ns0 in range(0, 4, 2):
                vps = [scr_pool.tile([128, 512], F32, name=f"scr_v{u}", tag="scr")
                       for u in range(2)]
                for ci in range(6):
                    for u in range(2):
                        nc.tensor.matmul(vps[u][:, 0:384],
                                         xt_t[:, ci, bass.ts(ns0 + u, 128)],
                                         wv_sb[:, ci, :],
                                         start=(ci == 0), stop=(ci == 5))
                for u in range(2):
                    jo = nt * 4 + ns0 + u
                    v_dst = v_sb[:, jo, :].rearrange("p (h c) -> p h c", c=65)[:, :, 0:64]
                    nc.vector.tensor_tensor(v_dst, vps[u][:, 0:384], bv_bc[:],
                                            mybir.AluOpType.add)

        # ---- phase A: Q/K for pair 0 only (attention starts ASAP; V and the
        # other pairs' Q/K are emitted inside the attention stream as filler)
        for nt in range(4):
            for _ in qk_chunk_gen(0, nt):
                pass

        # ---- attention; Q/K of pair p+1 and the output projection are
        # emitted between attention blocks as PE filler work ----
        def proj_block_gen(q5):
            for q1 in range(4 * q5, 4 * q5 + 4):
                out_sb = outp.tile([128, DIM], F32, name="out_sb")
                pps = [scr_pool.tile([128, 512], F32, name=f"scr_p{u}", tag="scr")
                       for u in range(2)]
                for kp in range(NPAIR):
                    for oh in range(2):
                        nc.tensor.matmul(pps[oh][:, 0:384],
                                         at_pairs[kp][:, bass.ts(q1, 128)],
                                         pw_sb[:, kp, bass.ts(oh, 384)],
                                         start=(kp == 0), stop=(kp == NPAIR - 1))
                for oh in range(2):
                    os_ = bass.ts(oh, 384)
                    nc.vector.tensor_tensor(out_sb[:, os_], pps[oh][:, 0:384],
                                            pb_bc[:, os_], mybir.AluOpType.add)
                nc.sync.dma_start(out_d[bass.ts(q1, 128), :], out_sb[:])
                yield

        for p in range(NPAIR):
            for q5 in range(NQ5):
                qs = bass.ts(q5, 512)
                blk = p * NQ5 + q5
                pv_big = pv_pool.tile([128, 2, 512], F32, name="pv_big")
                # filler emission spread through the block (PE work to fill
                # the bubbles of the ACT-bound attention stream)
                if blk in (1, 2, 3, 4):       # QK pair 1, chunks 0-3
                    filler = qk_chunk_gen(1, blk - 1)
                elif blk in (5, 6, 7, 8):     # QK pair 2, chunks 0-3
                    filler = qk_chunk_gen(2, blk - 5)
                elif blk in (9, 10, 11):      # output projection blocks
                    filler = proj_block_gen(blk - 9)
                else:
                    filler = iter(())
                for j in range(NJ):
                    if blk == 0 and j % 4 == 0:
                        v_chunk(j // 4)
                    if j in (4, 12):
                        next(filler, None)
                        next(filler, None)
                    # one ST tile holds both heads for key-tile j; the single
                    # tag double-buffers in 4 PSUM banks so scores of j+1 run
                    # during exp of j, and the two heads' score matmuls are
                    # adjacent (row-group packed, concurrent on the PE).
                    st = st_pool.tile([128, 2, 512], F32, name="st")
                    for h in range(2):
                        hs = slice(h * HD, (h + 1) * HD)
                        nc.tensor.matmul(st[:, h, :],
                                         kt_pairs[p][hs, bass.ts(j, 128)],
                                         qt_pairs[p][hs, qs],
                                         start=True, stop=True)
                    ptile = pt_pool.tile([128, 2, 512], F16, name="pt")
                    nc.scalar.activation(ptile[:], st[:],
                                         mybir.ActivationFunctionType.Exp)
                    for h in range(2):
                        hc = (2 * p + h) * 65
                        nc.tensor.matmul(pv_big[0:65, h, :],
                                         v_sb[:, j, hc:hc + 65],
                                         ptile[:, h, :],
                                         start=(j == 0), stop=(j == NJ - 1))
                for _ in filler:
                    pass
                # copy PV psum to SBUF right away (frees the PSUM tile)
                pv_sb = pvs_pool.tile([65, 2, 512], F32, name="pv_sb")
                nc.vector.tensor_copy(out=pv_sb[:], in_=pv_big[0:65, :, :])
                # denominators: scatter PSUM row 64 to a partition-0 [2,512]
                # tile (the custom-DVE reciprocal needs base_partition 0),
                # then reciprocal and selector broadcast.
                den2 = small.tile([2, 512], F32, name="den2")
                nc.sync.dma_start(den2[:], pv_sb[64:65, :, :])
                recip2 = small.tile([2, 512], F32, name="recip2")
                nc.vector.reciprocal_approx_fast(out=recip2[:], in_=den2[:])
                dbc = scr_pool.tile([128, 512], F32, name="scr")
                nc.tensor.matmul(dbc[:], sel[:], recip2[:],
                                 start=True, stop=True)
                for h in range(2):
                    hs = slice(h * HD, (h + 1) * HD)
                    nc.vector.tensor_tensor(at_pairs[p][hs, qs],
                                            pv_sb[0:64, h, :], dbc[hs, :],
                                            mybir.AluOpType.mult)

        for _ in proj_block_gen(3):
            pass

    nc.compile()
    return nc


def _get_nc():
    if "nc" not in _NC_CACHE:
        _NC_CACHE["nc"] = _build_nc()
    return _NC_CACHE["nc"]


def _install_ntff_shim():
    """Register the NTFF profile hook (missing antenv.axon_hooks in this image)."""
    import sys
    import types
    try:
        import antenv
        if "antenv.axon_hooks" in sys.modules:
            return
        mod = types.ModuleType("antenv.axon_hooks")
        state = {"hook": None}
        mod.set_axon_ntff_profile_hook = lambda h: state.__setitem__("hook", h)
        mod.get_axon_ntff_profile_hook = lambda: state["hook"]
        sys.modules["antenv.axon_hooks"] = mod
        antenv.axon_hooks = mod
        from trn_agent_boot.trn_boot import _ntff_profile_via_ctypes
        mod.set_axon_ntff_profile_hook(
            _ntff_profile_via_ctypes("/opt/axon/libaxon_pjrt.so"))
    except Exception:
        pass


def kernel(x, mask, qkv_w, qkv_b, proj_w, proj_b):
    global LAST_EXEC_TIME_NS
    x = np.asarray(x, dtype=np.float32)
    qkv_w = np.asarray(qkv_w, dtype=np.float32)
    qkv_b = np.asarray(qkv_b, dtype=np.float32)
    proj_w = np.asarray(proj_w, dtype=np.float32)
    proj_b = np.asarray(proj_b, dtype=np.float32)
    # mask is all-ones per the problem spec; softmax over the full key axis.

    sel = np.zeros((2, 128), np.float32)
    sel[0, 0:64] = 1.0
    sel[1, 64:128] = 1.0
    ones1 = np.ones((1, 128), np.float32)

    in_maps = []
    for c in range(8):
        b, g = divmod(c, 2)
        r0 = g * 384
        qr = slice(r0, r0 + 384)
        kr = slice(DIM + r0, DIM + r0 + 384)
        vr = slice(2 * DIM + r0, 2 * DIM + r0 + 384)
        in_maps.append({
            "xt": np.ascontiguousarray(x[b].T),
            "wq": np.ascontiguousarray((qkv_w[qr] * SCALE).T),
            "wk": np.ascontiguousarray(qkv_w[kr].T),
            "wv": np.ascontiguousarray(qkv_w[vr].T),
            "bq": np.ascontiguousarray(qkv_b[qr] * SCALE),
            "bk": np.ascontiguousarray(qkv_b[kr]),
            "bv": np.ascontiguousarray(qkv_b[vr])[None, :],
            "pw": np.ascontiguousarray(proj_w[:, r0:r0 + 384].T),
            "pb": (proj_b if g == 0 else np.zeros_like(proj_b))[None, :],
            "sel": sel,
            "ones1": ones1,
        })

    trace = os.environ.get("MHA_KERNEL_TRACE", "") == "1"
    if trace:
        _install_ntff_shim()
    nc = _get_nc()
    res = run_bass_kernel_spmd(nc, in_maps, list(range(8)), trace=trace)
    LAST_EXEC_TIME_NS = res.exec_time_ns

    out = np.empty((B, N, DIM), np.float32)
    for b in range(B):
        out[b] = res.results[2 * b]["out"] + res.results[2 * b + 1]["out"]
    return out


# revision 16
# speedup vs baseline: 1.8424x; 1.8424x over previous
"""Multi-head attention (B=4, N=2048, DIM=768, H=12) on 8 TRN2 NeuronCores.

Sharding: core c handles batch b = c//2 and head group g = c%2 (6 heads each).
Each core computes qkv projection, attention and the partial output projection
for its 6 heads; the host sums the two partial projections per batch (row-split
of the projection over heads). proj bias is applied on even cores only (odd
cores get zeros).

On-device dataflow (per core):
  - x^T is provided by the host as [768, 2048]; all matmuls run with the
    contraction on partitions.
  - Q^T / K^T are produced in [head_dim, n] layout, two heads packed per
    128-partition tile; scores are computed transposed (keys on partitions,
    queries on the free dim) so exp(scores) tiles feed the PV matmul directly
    with no transposes.
  - V is produced in natural [n, head_dim] layout, interleaved with a ones
    column per head; the ones column makes the PV matmul also accumulate the
    softmax denominator (row 64 of its PSUM output).
  - Softmax skips the max subtraction: scores here are ~N(0, 0.31) so exp
    needs no stabilization; matches jax softmax to fp32r/fp16 precision.
  - Normalization: PV PSUM is copied to SBUF immediately (releases the PSUM
    bank for the next tile), reciprocal_approx_fast on the denominators,
    broadcast across partitions via a K=2 selector matmul, multiply on DVE.
  - Program order interleaves the next pair's Q/K projection (and the output
    projection) into the ACT-bound attention stream so the PE stays busy and
    the HAM clock-gate keeps the PE at 2.4 GHz.
  - Matmuls run in float32r (1 cycle/row at N>=256, ~1.6e-4 rel err);
    probabilities and V in fp16 (~3e-4 contribution).
"""
import os
import numpy as np
from contextlib import ExitStack

import concourse.bass as bass
import concourse.tile as tile
from concourse import bacc, mybir
from concourse.bass_utils import run_bass_kernel_spmd

F32 = mybir.dt.float32
F32R = mybir.dt.float32r
F16 = mybir.dt.float16

B, N, DIM = 4, 2048, 768
H, HD = 12, 64
SCALE = HD ** -0.5
HPC = 6            # heads per core
NPAIR = 3          # head pairs per core
NJ = N // 128      # 16 key tiles
NQ5 = N // 512     # 4 query tiles of 512
JB = 2             # j-tiles per exp batch

_NC_CACHE = {}
LAST_EXEC_TIME_NS = None


def _build_nc():
    nc = bacc.Bacc("TRN2", target_bir_lowering=False, num_devices=1)

    xt_d = nc.declare_dram_parameter("xt", [DIM, N], F32R, isOutput=False)
    wq_d = nc.declare_dram_parameter("wq", [DIM, 384], F32R, isOutput=False)
    wk_d = nc.declare_dram_parameter("wk", [DIM, 384], F32R, isOutput=False)
    wv_d = nc.declare_dram_parameter("wv", [DIM, 384], F32R, isOutput=False)
    bq_d = nc.declare_dram_parameter("bq", [384], F32, isOutput=False)
    bk_d = nc.declare_dram_parameter("bk", [384], F32, isOutput=False)
    bv_d = nc.declare_dram_parameter("bv", [1, 384], F32R, isOutput=False)
    pw_d = nc.declare_dram_parameter("pw", [384, DIM], F32R, isOutput=False)
    pb_d = nc.declare_dram_parameter("pb", [1, DIM], F32R, isOutput=False)
    sel_d = nc.declare_dram_parameter("sel", [2, 128], F32, isOutput=False)
    ones_d = nc.declare_dram_parameter("ones1", [1, 128], F32R, isOutput=False)
    out_d = nc.declare_dram_parameter("out", [N, DIM], F32, isOutput=True)

    with tile.TileContext(nc) as tc, ExitStack() as ctx:
        consts = ctx.enter_context(tc.tile_pool(name="consts", bufs=1))
        xt_pool = ctx.enter_context(tc.tile_pool(name="xtp", bufs=2))
        big = ctx.enter_context(tc.tile_pool(name="big", bufs=1))
        pt_pool = ctx.enter_context(tc.tile_pool(name="ptp", bufs=4))
        pvs_pool = ctx.enter_context(tc.tile_pool(name="pvsp", bufs=2))
        small = ctx.enter_context(tc.tile_pool(name="small", bufs=2))
        outp = ctx.enter_context(tc.tile_pool(name="outp", bufs=3))
        st_pool = ctx.enter_context(tc.tile_pool(name="stp", bufs=2, space="PSUM"))
        pv_pool = ctx.enter_context(tc.tile_pool(name="pvp", bufs=1, space="PSUM"))
        scr_pool = ctx.enter_context(tc.tile_pool(name="scrp", bufs=2, space="PSUM"))

        # ---- constants ----
        wq_sb = consts.tile([128, 6, 384], F32R)
        wk_sb = consts.tile([128, 6, 384], F32R)
        wv_sb = consts.tile([128, 6, 384], F32R)
        nc.sync.dma_start(wq_sb[:], wq_d.rearrange("(co ci) m -> ci co m", ci=128))
        nc.sync.dma_start(wk_sb[:], wk_d.rearrange("(co ci) m -> ci co m", ci=128))
        nc.sync.dma_start(wv_sb[:], wv_d.rearrange("(co ci) m -> ci co m", ci=128))
        pw_sb = consts.tile([128, 3, DIM], F32R)
        nc.sync.dma_start(pw_sb[:], pw_d.rearrange("(ko ki) o -> ki ko o", ki=128))
        bq_sb = consts.tile([128, 3], F32)
        bk_sb = consts.tile([128, 3], F32)
        nc.sync.dma_start(bq_sb[:], bq_d.rearrange("(po pi) -> pi po", pi=128))
        nc.sync.dma_start(bk_sb[:], bk_d.rearrange("(po pi) -> pi po", pi=128))
        bv1 = consts.tile([1, 384], F32R)
        pb1 = consts.tile([1, DIM], F32R)
        nc.sync.dma_start(bv1[:], bv_d[:])
        nc.sync.dma_start(pb1[:], pb_d[:])
        sel = consts.tile([2, 128], F32)
        ones1 = consts.tile([1, 128], F32R)
        nc.sync.dma_start(sel[:], sel_d[:])
        nc.sync.dma_start(ones1[:], ones_d[:])

        # broadcast bv/pb across partitions via K=1 ones matmul
        bv_bc = consts.tile([128, 384], F32)
        pb_bc = consts.tile([128, DIM], F32)
        bv_ps = scr_pool.tile([128, 512], F32, name="scr")
        nc.tensor.matmul(bv_ps[:, 0:384], ones1[:], bv1[:], start=True, stop=True)
        nc.vector.tensor_copy(out=bv_bc[:], in_=bv_ps[:, 0:384])
        for oh in range(2):
            pb_ps = scr_pool.tile([128, 512], F32, name="scr")
            nc.tensor.matmul(pb_ps[:, 0:384], ones1[:], pb1[:, bass.ts(oh, 384)],
                             start=True, stop=True)
            nc.vector.tensor_copy(out=pb_bc[:, bass.ts(oh, 384)], in_=pb_ps[:, 0:384])

        qt_pairs = [big.tile([128, N], F16, name=f"qt{p}") for p in range(NPAIR)]
        kt_pairs = [big.tile([128, N], F16, name=f"kt{p}") for p in range(NPAIR)]
        v_sb = big.tile([128, NJ, HPC * 65], F16)
        nc.vector.memset(v_sb[:], 1.0)  # ones columns (at h*65+64) survive
        at_pairs = [big.tile([128, N], F32R, name=f"at{p}") for p in range(NPAIR)]

        def qk_chunk_gen(p, nt):
            """Q and K projection for pair p on the 512-wide n-chunk nt.
            Generator yielding between small pieces so the emission can be
            spread through an ACT-bound attention block. Q and K matmuls are
            interleaved so consecutive matmuls hit different PSUM banks."""
            ns_ = bass.ts(nt, 512)
            xt_t = load_xt(nt)
            qp = scr_pool.tile([128, 512], F32, name="scr")
            for ci in range(6):
                nc.tensor.matmul(qp[:], wq_sb[:, ci, bass.ts(p, 128)],
                                 xt_t[:, ci, :], start=(ci == 0), stop=(ci == 5))
                if ci == 2:
                    yield
            nc.vector.tensor_scalar_add(qt_pairs[p][:, ns_], qp[:], bq_sb[:, p:p + 1])
            yield
            kp = scr_pool.tile([128, 512], F32, name="scr")
            for ci in range(6):
                nc.tensor.matmul(kp[:], wk_sb[:, ci, bass.ts(p, 128)],
                                 xt_t[:, ci, :], start=(ci == 0), stop=(ci == 5))
                if ci == 2:
                    yield
            nc.vector.tensor_scalar_add(kt_pairs[p][:, ns_], kp[:], bk_sb[:, p:p + 1])
            yield

        def load_xt(nt):
            xt_t = xt_pool.tile([128, 6, 512], F32R, name="xt_t")
            nc.sync.dma_start(
                xt_t[:], xt_d.rearrange("(co ci) n -> ci co n", ci=128)[:, :, bass.ts(nt, 512)]
            )
            return xt_t

        def v_chunk(nt):
            xt_t = load_xt(nt)
            for ns0 in range(0, 4, 2):
                vps = [scr_pool.tile([128, 512], F32, name=f"scr_v{u}", tag="scr")
                       for u in range(2)]
                for ci in range(6):
                    for u in range(2):
                        nc.tensor.matmul(vps[u][:, 0:384],
                                         xt_t[:, ci, bass.ts(ns0 + u, 128)],
                                         wv_sb[:, ci, :],
                                         start=(ci == 0), stop=(ci == 5))
                for u in range(2):
                    jo = nt * 4 + ns0 + u
                    v_dst = v_sb[:, jo, :].rearrange("p (h c) -> p h c", c=65)[:, :, 0:64]
                    nc.vector.tensor_tensor(v_dst, vps[u][:, 0:384], bv_bc[:],
                                            mybir.AluOpType.add)

        # ---- phase A: Q/K for pair 0 only (attention starts ASAP; V and the
        # other pairs' Q/K are emitted inside the attention stream as filler)
        for nt in range(4):
            for _ in qk_chunk_gen(0, nt):
                pass

        # ---- attention; Q/K of pair p+1 and the output projection are
        # emitted between attention blocks as PE filler work ----
        def proj_block_gen(q5):
            for q1 in range(4 * q5, 4 * q5 + 4):
                out_sb = outp.tile([128, DIM], F32, name="out_sb")
                pps = [scr_pool.tile([128, 512], F32, name=f"scr_p{u}", tag="scr")
                       for u in range(2)]
                for kp in range(NPAIR):
                    for oh in range(2):
                        nc.tensor.matmul(pps[oh][:, 0:384],
                                         at_pairs[kp][:, bass.ts(q1, 128)],
                                         pw_sb[:, kp, bass.ts(oh, 384)],
                                         start=(kp == 0), stop=(kp == NPAIR - 1))
                for oh in range(2):
                    os_ = bass.ts(oh, 384)
                    nc.vector.tensor_tensor(out_sb[:, os_], pps[oh][:, 0:384],
                                            pb_bc[:, os_], mybir.AluOpType.add)
                nc.sync.dma_start(out_d[bass.ts(q1, 128), :], out_sb[:])
                yield

        for p in range(NPAIR):
            for q5 in range(NQ5):
                qs = bass.ts(q5, 512)
                blk = p * NQ5 + q5
                pv_big = pv_pool.tile([128, 2, 512], F32, name="pv_big")
                # filler emission spread through the block (PE work to fill
                # the bubbles of the ACT-bound attention stream)
                if blk in (1, 2, 3, 4):       # QK pair 1, chunks 0-3
                    filler = qk_chunk_gen(1, blk - 1)
                elif blk in (5, 6, 7, 8):     # QK pair 2, chunks 0-3
                    filler = qk_chunk_gen(2, blk - 5)
                elif blk in (9, 10, 11):      # output projection blocks
                    filler = proj_block_gen(blk - 9)
                else:
                    filler = iter(())
                for j in range(NJ):
                    if blk == 0 and j % 4 == 0:
                        v_chunk(j // 4)
                    if j in (4, 12):
                        next(filler, None)
                        next(filler, None)
                    # one ST tile holds both heads for key-tile j; the single
                    # tag double-buffers in 4 PSUM banks so scores of j+1 run
                    # during exp of j, and the two heads' score matmuls are
                    # adjacent (row-group packed, concurrent on the PE).
                    st = st_pool.tile([128, 2, 512], F32, name="st")
                    for h in range(2):
                        hs = slice(h * HD, (h + 1) * HD)
                        nc.tensor.matmul(st[:, h, :],
                                         kt_pairs[p][hs, bass.ts(j, 128)],
                                         qt_pairs[p][hs, qs],
                                         start=True, stop=True)
                    ptile = pt_pool.tile([128, 2, 512], F16, name="pt")
                    nc.scalar.activation(ptile[:], st[:],
                                         mybir.ActivationFunctionType.Exp)
                    for h in range(2):
                        hc = (2 * p + h) * 65
                        nc.tensor.matmul(pv_big[0:65, h, :],
                                         v_sb[:, j, hc:hc + 65],
                                         ptile[:, h, :],
                                         start=(j == 0), stop=(j == NJ - 1))
                for _ in filler:
                    pass
                # copy PV psum to SBUF right away (frees the PSUM tile)
                pv_sb = pvs_pool.tile([65, 2, 512], F32, name="pv_sb")
                nc.vector.tensor_copy(out=pv_sb[:], in_=pv_big[0:65, :, :])
                # denominators: scatter PSUM row 64 to a partition-0 [2,512]
                # tile (the custom-DVE reciprocal needs base_partition 0),
                # then reciprocal and selector broadcast.
                den2 = small.tile([2, 512], F32, name="den2")
                nc.sync.dma_start(den2[:], pv_sb[64:65, :, :])
                recip2 = small.tile([2, 512], F32, name="recip2")
                nc.vector.reciprocal_approx_fast(out=recip2[:], in_=den2[:])
                dbc = scr_pool.tile([128, 512], F32, name="scr")
                nc.tensor.matmul(dbc[:], sel[:], recip2[:],
                                 start=True, stop=True)
                for h in range(2):
                    hs = slice(h * HD, (h + 1) * HD)
                    nc.vector.tensor_tensor(at_pairs[p][hs, qs],
                                            pv_sb[0:64, h, :], dbc[hs, :],
                                            mybir.AluOpType.mult)

        for _ in proj_block_gen(3):
            pass

    nc.compile()
    return nc


def _get_nc():
    if "nc" not in _NC_CACHE:
        _NC_CACHE["nc"] = _build_nc()
    return _NC_CACHE["nc"]


def _install_ntff_shim():
    """Register the NTFF profile hook (missing antenv.axon_hooks in this image)."""
    import sys
    import types
    try:
        import antenv
        if "antenv.axon_hooks" in sys.modules:
            return
        mod = types.ModuleType("antenv.axon_hooks")
        state = {"hook": None}
        mod.set_axon_ntff_profile_hook = lambda h: state.__setitem__("hook", h)
        mod.get_axon_ntff_profile_hook = lambda: state["hook"]
        sys.modules["antenv.axon_hooks"] = mod
        antenv.axon_hooks = mod
        from trn_agent_boot.trn_boot import _ntff_profile_via_ctypes
        mod.set_axon_ntff_profile_hook(
            _ntff_profile_via_ctypes("/opt/axon/libaxon_pjrt.so"))
    except Exception:
        pass


def kernel(x, mask, qkv_w, qkv_b, proj_w, proj_b):
    global LAST_EXEC_TIME_NS
    x = np.asarray(x, dtype=np.float32)
    qkv_w = np.asarray(qkv_w, dtype=np.float32)
    qkv_b = np.asarray(qkv_b, dtype=np.float32)
    proj_w = np.asarray(proj_w, dtype=np.float32)
    proj_b = np.asarray(proj_b, dtype=np.float32)
    # mask is all-ones per the problem spec; softmax over the full key axis.

    sel = np.zeros((2, 128), np.float32)
    sel[0, 0:64] = 1.0
    sel[1, 64:128] = 1.0
    ones1 = np.ones((1, 128), np.float32)

    in_maps = []
    for c in range(8):
        b, g = divmod(c, 2)
        r0 = g * 384
        qr = slice(r0, r0 + 384)
        kr = slice(DIM + r0, DIM + r0 + 384)
        vr = slice(2 * DIM + r0, 2 * DIM + r0 + 384)
        in_maps.append({
            "xt": np.ascontiguousarray(x[b].T),
            "wq": np.ascontiguousarray((qkv_w[qr] * SCALE).T),
            "wk": np.ascontiguousarray(qkv_w[kr].T),
            "wv": np.ascontiguousarray(qkv_w[vr].T),
            "bq": np.ascontiguousarray(qkv_b[qr] * SCALE),
            "bk": np.ascontiguousarray(qkv_b[kr]),
            "bv": np.ascontiguousarray(qkv_b[vr])[None, :],
            "pw": np.ascontiguousarray(proj_w[:, r0:r0 + 384].T),
            "pb": (proj_b if g == 0 else np.zeros_like(proj_b))[None, :],
            "sel": sel,
            "ones1": ones1,
        })

    trace = os.environ.get("MHA_KERNEL_TRACE", "") == "1"
    if trace:
        _install_ntff_shim()
    nc = _get_nc()
    res = run_bass_kernel_spmd(nc, in_maps, list(range(8)), trace=trace)
    LAST_EXEC_TIME_NS = res.exec_time_ns

    out = np.empty((B, N, DIM), np.float32)
    for b in range(B):
        out[b] = res.results[2 * b]["out"] + res.results[2 * b + 1]["out"]
    return out
